# revision 11
# baseline (speedup 1.0000x reference)
"""Trainium2 Bass kernel for the DiscreteAgent GNN (NNConv + LN + MLP head).

Strategy (8 NeuronCores, SPMD, no collectives):
  * Edges are bucketed by destination node range: core c owns dst nodes
    [c*6250, (c+1)*6250) and receives exactly the edges pointing into that
    range.  Each core therefore produces its own disjoint slice of the
    output -> no all-reduce is required at all.
  * Within a core, edges are sorted by destination block (128 nodes per
    block) and padded to a fixed per-block capacity C.  segment_sum becomes,
    per node block, a chain of one-hot matmuls accumulated in PSUM
    (onehot[e, n_local]^T @ msg[e, :]), fully deterministic.
  * Host-side work is index/layout manipulation only (sort, pad, transpose,
    gather of x rows by edge_src); every FLOP runs on the device.

Dtype choices: PE matmuls on the edge path run in float32r (same bits as
f32, single-pass PE); the small node-phase MLP matmuls and the one-hot scatter run in fp16; LN and
reductions accumulate in f32 (PSUM is always f32).

Per-core device pipeline, per 128-edge tile (DMAs batched 8 tiles/group):
  PE:  w_pre = [edge_attr|1]^T @ [We;be]  (K=9, f32r) -> PSUM [128, 512]
  ACT: w_relu = relu(w_pre)               -> SBUF
  DVE/GPSIMD: prod = w_relu * broadcast(x_src)   ([128, 32, 16], i inner, fp16)
  DVE: onehot = (iota_row == dst_local)   -> [128, 128] (fp16 out)
  PE:  agg_exp_psum += onehot^T @ prod    (N=512 K-chain over block's tiles)
Per 128-node block:
  PE:  root = [x|1]^T @ [Wroot;bconv] (f32r); DVE: h = reduce_i(agg_exp)+root
  DVE/ACT: LayerNorm + relu
  PE:  featT = Wlin^T @ hreluT ; q1T = Wq1^T @ featT ; qT = Wq2^T @ q1rT
       (fp16, transposed layout; biases folded into aug rows / ACT bias)
  out: qT slice [32, 128] -> DRAM
"""

import numpy as np

# ---- problem constants (hardcoded per contract) ----
N = 50000
E = 200000
IN_C = 16
HID_C = 32
EDGE_D = 8
OUT_C = 32
MLP_H = 128
N_ACT = 32

M = 8                 # cores
P = 128               # partitions
NPC = N // M          # 6250 nodes per core
NB = (NPC + P - 1) // P   # 49 blocks per core
NPC_PAD = NB * P      # 6272
G = 8                 # edge tiles per DMA group

_PROGRAM_CACHE: dict = {}


def _build_program(C: int, gpsimd_frac: int):
    """Build + compile the SPMD Bass program for per-block edge capacity C.

    gpsimd_frac: out of 4 edge tiles, how many run their broadcast-multiply
    on the GPSIMD engine instead of DVE (load balancing).
    """
    import concourse.tile as tile
    from concourse import bacc, mybir
    from concourse.masks import make_identity

    f32 = mybir.dt.float32
    f32r = mybir.dt.float32r
    fp16 = mybir.dt.float16
    i32 = mybir.dt.int32
    KT = C // P             # K-tiles (edge tiles) per node block
    ET = NB * KT            # edge tiles per core
    EPC = NB * C            # padded edge slots per core

    nc = bacc.Bacc("TRN2", target_bir_lowering=False, debug=False, num_devices=M)

    # --- DRAM I/O (per core) ---
    attrT = nc.dram_tensor("attrT", [EDGE_D + 1, EPC], f32r, kind="ExternalInput")
    xjg = nc.dram_tensor("xjg", [EPC, IN_C], fp16, kind="ExternalInput")
    dstl = nc.dram_tensor("dstl", [EPC], f32, kind="ExternalInput")
    xsT = nc.dram_tensor("xsT", [IN_C + 1, NPC_PAD], f32r, kind="ExternalInput")
    weA = nc.dram_tensor("weA", [EDGE_D + 1, IN_C * HID_C], f32r, kind="ExternalInput")
    wrootA = nc.dram_tensor("wrootA", [IN_C + 1, HID_C], f32r, kind="ExternalInput")
    wlin = nc.dram_tensor("wlin", [HID_C, OUT_C], fp16, kind="ExternalInput")
    wq1 = nc.dram_tensor("wq1", [OUT_C, MLP_H], fp16, kind="ExternalInput")
    wq2 = nc.dram_tensor("wq2", [MLP_H, N_ACT], fp16, kind="ExternalInput")
    bq1c = nc.dram_tensor("bq1c", [MLP_H, 1], f32, kind="ExternalInput")
    bq2c = nc.dram_tensor("bq2c", [N_ACT, 1], f32, kind="ExternalInput")
    gammab = nc.dram_tensor("gammab", [P, HID_C], f32, kind="ExternalInput")
    betab = nc.dram_tensor("betab", [P, HID_C], f32, kind="ExternalInput")
    qT = nc.dram_tensor("qT", [N_ACT, NPC_PAD], f32, kind="ExternalOutput")

    with tile.TileContext(nc) as tc:
        with (
            tc.tile_pool(name="const", bufs=1) as cpool,
            tc.tile_pool(name="edge_in", bufs=3) as epool,
            tc.tile_pool(name="wrelu", bufs=4) as wpool,
            tc.tile_pool(name="work", bufs=4) as kpool,
            tc.tile_pool(name="node", bufs=3) as npool,
            tc.tile_pool(name="wpre_ps", bufs=2, space="PSUM") as wpre_ps,
            tc.tile_pool(name="agg_ps", bufs=2, space="PSUM") as agg_ps,
            tc.tile_pool(name="node_ps", bufs=4, space="PSUM") as node_ps,
        ):
            # ---- persistent constants in SBUF ----
            we_sb = cpool.tile([EDGE_D + 1, IN_C * HID_C], f32r, tag="we")
            nc.sync.dma_start(we_sb[:], weA.ap()[:])
            xsT_sb = cpool.tile([IN_C + 1, NPC_PAD], f32r, tag="xsT")
            nc.sync.dma_start(xsT_sb[:], xsT.ap()[:])
            wroot_sb = cpool.tile([IN_C + 1, HID_C], f32r, tag="wroot")
            nc.sync.dma_start(wroot_sb[:], wrootA.ap()[:])
            wlin_sb = cpool.tile([HID_C, OUT_C], fp16, tag="wlin")
            nc.sync.dma_start(wlin_sb[:], wlin.ap()[:])
            wq1_sb = cpool.tile([OUT_C, MLP_H], fp16, tag="wq1")
            nc.sync.dma_start(wq1_sb[:], wq1.ap()[:])
            wq2_sb = cpool.tile([MLP_H, N_ACT], fp16, tag="wq2")
            nc.sync.dma_start(wq2_sb[:], wq2.ap()[:])
            bq1_sb = cpool.tile([MLP_H, 1], f32, tag="bq1")
            nc.sync.dma_start(bq1_sb[:], bq1c.ap()[:])
            bq2_sb = cpool.tile([N_ACT, 1], f32, tag="bq2")
            nc.sync.dma_start(bq2_sb[:], bq2c.ap()[:])
            gamma_sb = cpool.tile([P, HID_C], f32, tag="gamma")
            nc.sync.dma_start(gamma_sb[:], gammab.ap()[:])
            beta_sb = cpool.tile([P, HID_C], f32, tag="beta")
            nc.sync.dma_start(beta_sb[:], betab.ap()[:])

            # iota row constant: every partition holds [0, 1, ..., 127]
            iota_i = cpool.tile([P, P], i32, tag="iota_i")
            nc.gpsimd.iota(iota_i[:], pattern=[[1, P]], base=0, channel_multiplier=0)
            iota_f = cpool.tile([P, P], fp16, tag="iota_f")
            nc.vector.tensor_copy(iota_f[:], iota_i[:])
            # identity for PE transpose
            ident = cpool.tile([P, P], f32, tag="ident")
            make_identity(nc, ident[:])
            # layernorm epsilon as a per-partition scalar const
            eps_c = cpool.tile([P, 1], f32, tag="eps")
            nc.gpsimd.memset(eps_c[:], 1e-5)

            agg_cur = None
            attr_g = xj_g = dstl_g = None
            for t in range(ET):
                b, kt = divmod(t, KT)
                g, tt = divmod(t, G)

                if tt == 0:
                    # batched loads for the next G edge tiles
                    gs = min(G, ET - g * G)
                    esl = slice(g * G * P, (g * G + gs) * P)
                    attr_g = epool.tile([EDGE_D + 1, G * P], f32r, tag="attr")
                    nc.sync.dma_start(attr_g[:, :gs * P], attrT.ap()[:, esl])
                    xj_g = epool.tile([P, G, IN_C], fp16, tag="xj")
                    nc.sync.dma_start(
                        xj_g[:, :gs, :],
                        xjg.ap()[esl, :].rearrange("(tt p) i -> p tt i", p=P))
                    dstl_g = epool.tile([P, G], f32, tag="dstl")
                    nc.sync.dma_start(
                        dstl_g[:, :gs],
                        dstl.ap()[esl, None].rearrange("(tt p) o -> p (tt o)", p=P))

                # w_pre = [attr|1]^T @ [We;be] -> PSUM [128, 512] (f32r)
                wpre = wpre_ps.tile([P, IN_C * HID_C], f32, tag="wpre")
                nc.tensor.matmul(wpre[:], lhsT=attr_g[:, tt * P:(tt + 1) * P],
                                 rhs=we_sb[:], start=True, stop=True)
                # relu -> SBUF
                wrelu = wpool.tile([P, IN_C * HID_C], fp16, tag="wrelu")
                nc.scalar.activation(wrelu[:], wpre[:],
                                     mybir.ActivationFunctionType.Relu)
                # prod = wrelu * broadcast(xj); layout [p, (h, i)] -> i inner
                prod = wpool.tile([P, IN_C * HID_C], fp16, tag="prod")
                xj_b = xj_g[:, tt, :].unsqueeze(1).to_broadcast([P, HID_C, IN_C])
                prod_3d = prod[:].rearrange("p (h i) -> p h i", h=HID_C)
                wrelu_3d = wrelu[:].rearrange("p (h i) -> p h i", h=HID_C)
                if t % 4 < gpsimd_frac:
                    nc.gpsimd.tensor_tensor(prod_3d, wrelu_3d, xj_b,
                                            op=mybir.AluOpType.mult)
                else:
                    nc.vector.tensor_tensor(prod_3d, wrelu_3d, xj_b,
                                            op=mybir.AluOpType.mult)
                # msg = sum_i prod  -> [128, 32] (f32r out for the PE)
                msg = kpool.tile([P, HID_C], fp16, tag="msg")
                with nc.allow_low_precision(reason="16-term reduce; fp16 msg"):
                    nc.vector.tensor_reduce(
                        msg[:], prod[:].rearrange("p (h i) -> p h i", h=HID_C),
                        axis=mybir.AxisListType.X, op=mybir.AluOpType.add)
                # onehot[e, n_local] = (iota_row == dst_local[e])  (f32r out)
                onehot = kpool.tile([P, P], fp16, tag="onehot")
                nc.vector.tensor_scalar(
                    onehot[:], iota_f[:], dstl_g[:, tt:tt + 1], None,
                    op0=mybir.AluOpType.is_equal)

                # scatter the full 512-wide prod: agg_exp += onehot^T @ prod
                if kt == 0:
                    agg_cur = agg_ps.tile([P, IN_C * HID_C], f32, tag="agg")
                nc.tensor.matmul(agg_cur[:], lhsT=onehot[:], rhs=prod[:],
                                 start=(kt == 0), stop=(kt == KT - 1))

                if kt == KT - 1:
                    # ---- node phase for block b ----
                    nsl = slice(b * P, (b + 1) * P)
                    # root term: [x|1]^T @ [Wroot;bconv] in its own PSUM
                    r_full = node_ps.tile([MLP_H, P], f32, tag="nps")
                    root_ps = r_full[:, :HID_C]
                    nc.tensor.matmul(root_ps[:], lhsT=xsT_sb[:, nsl],
                                     rhs=wroot_sb[:], start=True, stop=True)
                    # reduce agg_exp over i -> h0, then h = h0 + root
                    h0 = npool.tile([P, HID_C], f32, tag="h0")
                    nc.vector.tensor_reduce(
                        h0[:], agg_cur[:].rearrange("p (h i) -> p h i", h=HID_C),
                        axis=mybir.AxisListType.X, op=mybir.AluOpType.add)
                    hfull = npool.tile([P, HID_C], f32, tag="hfull")
                    nc.vector.tensor_add(hfull[:], h0[:], root_ps[:])
                    # LayerNorm over HID_C
                    musum = npool.tile([P, 1], f32, tag="musum")
                    nc.vector.tensor_reduce(musum[:], hfull[:],
                                            axis=mybir.AxisListType.X,
                                            op=mybir.AluOpType.add)
                    negmu = npool.tile([P, 1], f32, tag="negmu")
                    nc.scalar.mul(negmu[:], musum[:], -1.0 / HID_C)
                    hc = npool.tile([P, HID_C], f32, tag="hc")
                    nc.vector.tensor_scalar(hc[:], hfull[:], negmu[:, :1], None,
                                            op0=mybir.AluOpType.add)
                    sq = npool.tile([P, HID_C], f32, tag="sq")
                    varsum = npool.tile([P, 1], f32, tag="varsum")
                    nc.scalar.activation(sq[:], hc[:],
                                         mybir.ActivationFunctionType.Square,
                                         accum_out=varsum[:])
                    std = npool.tile([P, 1], f32, tag="std")
                    nc.scalar.activation(std[:], varsum[:],
                                         mybir.ActivationFunctionType.Sqrt,
                                         scale=1.0 / HID_C, bias=eps_c[:, :1])
                    rstd = npool.tile([P, 1], f32, tag="rstd")
                    nc.vector.reciprocal(rstd[:], std[:])
                    # (hc * rstd) * gamma
                    t2 = npool.tile([P, HID_C], f32, tag="t2")
                    nc.vector.scalar_tensor_tensor(
                        t2[:], hc[:], rstd[:, :1], gamma_sb[:],
                        op0=mybir.AluOpType.mult, op1=mybir.AluOpType.mult)
                    t3 = npool.tile([P, HID_C], f32, tag="t3")
                    nc.vector.tensor_add(t3[:], t2[:], beta_sb[:])
                    hrelu = npool.tile([P, HID_C], f32, tag="hrelu")
                    nc.scalar.activation(hrelu[:], t3[:],
                                         mybir.ActivationFunctionType.Relu)
                    # transpose hrelu -> [32, 128]
                    hT_full = node_ps.tile([MLP_H, P], f32, tag="nps")
                    hT_ps = hT_full[:HID_C]
                    nc.tensor.transpose(hT_ps[:], hrelu[:], ident[:])
                    hT = npool.tile([HID_C, P], fp16, tag="hTs")
                    nc.scalar.copy(hT[:], hT_ps[:])
                    # featT = Wlin^T @ hT   [32, 128]  (bf16)
                    fT_full = node_ps.tile([MLP_H, P], f32, tag="nps")
                    fT_ps = fT_full[:OUT_C]
                    nc.tensor.matmul(fT_ps[:], lhsT=wlin_sb[:], rhs=hT[:],
                                     start=True, stop=True)
                    fT = npool.tile([OUT_C, P], fp16, tag="fTs")
                    nc.scalar.copy(fT[:], fT_ps[:])
                    # q1T = Wq1^T @ featT  [128, 128]; bias+relu via ACT (bf16)
                    q1_ps = node_ps.tile([MLP_H, P], f32, tag="nps")
                    nc.tensor.matmul(q1_ps[:], lhsT=wq1_sb[:], rhs=fT[:],
                                     start=True, stop=True)
                    q1r = npool.tile([MLP_H, P], fp16, tag="q1r")
                    nc.scalar.activation(q1r[:], q1_ps[:],
                                         mybir.ActivationFunctionType.Relu,
                                         bias=bq1_sb[:, :1])
                    # qT = Wq2^T @ q1rT  [32, 128]
                    q_full = node_ps.tile([MLP_H, P], f32, tag="nps")
                    q_ps = q_full[:N_ACT]
                    nc.tensor.matmul(q_ps[:], lhsT=wq2_sb[:], rhs=q1r[:],
                                     start=True, stop=True)
                    qfin = npool.tile([N_ACT, P], f32, tag="qfin")
                    nc.vector.tensor_scalar(qfin[:], q_ps[:], bq2_sb[:, :1], None,
                                            op0=mybir.AluOpType.add)
                    nc.sync.dma_start(qT.ap()[:, nsl], qfin[:])

    nc.compile()
    return nc


def _get_program(C: int, gpsimd_frac: int):
    key = (C, gpsimd_frac)
    if key not in _PROGRAM_CACHE:
        _PROGRAM_CACHE[key] = _build_program(C, gpsimd_frac)
    return _PROGRAM_CACHE[key]


def _prep_inputs(x, edge_src, edge_dst, edge_attr,
                 We, be, Wroot, bconv, gamma, beta,
                 Wlin, blin, Wq1, bq1, Wq2, bq2):
    """Host-side sharding: bucket+sort edges by destination, pad per block,
    build per-core input maps. Index/layout work only."""
    f32 = np.float32
    x = np.asarray(x, f32)
    edge_src = np.asarray(edge_src)
    edge_dst = np.asarray(edge_dst)
    edge_attr = np.asarray(edge_attr, f32)

    order = np.argsort(edge_dst, kind="stable")
    dst_s = edge_dst[order]
    src_s = edge_src[order]
    attr_s = edge_attr[order]

    core_of = dst_s // NPC
    local = dst_s - core_of * NPC
    blk = local // P
    gblk = core_of * NB + blk
    counts = np.bincount(gblk, minlength=M * NB)
    C = int(max(256, -(-counts.max() // P) * P))
    EPC = NB * C

    starts = np.zeros(M * NB, np.int64)
    starts[1:] = np.cumsum(counts)[:-1]
    rank = np.arange(E, dtype=np.int64) - starts[gblk]
    slot = gblk.astype(np.int64) * C + rank    # into [M*NB*C]

    tot = M * NB * C
    attr_all = np.zeros((tot, EDGE_D + 1), f32)
    attr_all[slot, :EDGE_D] = attr_s
    attr_all[slot, EDGE_D] = 1.0
    xj_all = np.zeros((tot, IN_C), np.float16)
    xj_all[slot] = x[src_s].astype(np.float16)
    dstl_all = np.full(tot, -1.0, f32)
    dstl_all[slot] = (local % P).astype(f32)

    attr_all = attr_all.reshape(M, EPC, EDGE_D + 1)
    xj_all = xj_all.reshape(M, EPC, IN_C)
    dstl_all = dstl_all.reshape(M, EPC)

    # node-slice features, augmented with ones row, transposed
    x_pad = np.zeros((M, NPC_PAD, IN_C + 1), f32)
    x_resh = x.reshape(M, NPC, IN_C)
    x_pad[:, :NPC, :IN_C] = x_resh
    x_pad[:, :, IN_C] = 1.0

    # parameters (replicated)
    We = np.asarray(We, f32)
    be = np.asarray(be, f32)
    Wroot = np.asarray(Wroot, f32)
    bconv = np.asarray(bconv, f32)
    gamma = np.asarray(gamma, f32)
    beta = np.asarray(beta, f32)
    Wlin = np.asarray(Wlin, f32)
    blin = np.asarray(blin, f32)
    Wq1 = np.asarray(Wq1, f32)
    bq1 = np.asarray(bq1, f32)
    Wq2 = np.asarray(Wq2, f32)
    bq2 = np.asarray(bq2, f32)

    weA = np.concatenate([We, be[None, :]], axis=0)            # [9, 512]
    # permute columns from (i, h) to (h, i) layout so the device-side
    # i-reduction is over the contiguous innermost dim
    weA_perm = np.ascontiguousarray(
        weA.reshape(EDGE_D + 1, IN_C, HID_C).transpose(0, 2, 1)
           .reshape(EDGE_D + 1, IN_C * HID_C))
    wrootA = np.concatenate([Wroot, bconv[None, :]], axis=0)   # [17, 32]
    bq1p = (blin @ Wq1 + bq1).astype(f32)                      # blin folded
    gammab = np.broadcast_to(gamma, (P, HID_C)).copy()
    betab = np.broadcast_to(beta, (P, HID_C)).copy()

    in_maps = []
    for c in range(M):
        in_maps.append({
            "attrT": np.ascontiguousarray(attr_all[c].T),
            "xjg": np.ascontiguousarray(xj_all[c]),
            "dstl": np.ascontiguousarray(dstl_all[c]),
            "xsT": np.ascontiguousarray(x_pad[c].T),
            "weA": weA_perm,
            "wrootA": wrootA,
            "wlin": Wlin.astype(np.float16),
            "wq1": Wq1.astype(np.float16),
            "wq2": Wq2.astype(np.float16),
            "bq1c": bq1p[:, None],
            "bq2c": bq2[:, None],
            "gammab": gammab,
            "betab": betab,
        })
    return C, in_maps


GPSIMD_FRAC = 3  # of every 4 edge tiles, how many multiply on GPSIMD


def kernel(**inputs) -> np.ndarray:
    from concourse.bass_utils import run_bass_kernel_spmd

    C, in_maps = _prep_inputs(**inputs)
    nc = _get_program(C, GPSIMD_FRAC)
    res = run_bass_kernel_spmd(nc, in_maps, list(range(M)))
    q = np.empty((N, N_ACT), np.float32)
    for c in range(M):
        q[c * NPC:(c + 1) * NPC] = res.results[c]["qT"][:, :NPC].T
    return q
